# revision 3
# baseline (speedup 1.0000x reference)
"""FlowNet-style correlation layer (MAX_DISPLACEMENT=4, 81 channels) on 8 TRN2 cores.

Strategy
--------
Data-parallel over batch N=8 -> 1 sample per NeuronCore.

Per core, for each 8x16 spatial patch of data1 (the "stationary" block, M=128
positions) we matmul against the surrounding 16x24 patch of zero-padded data2
(the "moving" block, N=384 columns), contracting over C=256 in two K=128
chunks accumulated in PSUM.  The [128, 16, 24] PSUM tile contains, for every
stationary position m=(g,j), all 81 correlation values at (t=g+dy, u=j+dx).

v2 changes vs the original band kernel (which was DMA-bound):
  * data2 is loaded as ONE padded [C, 104, 168] image per core (fully
    contiguous DMA) instead of 10 overlapping x-slabs (48B-run DMAs at half
    bandwidth, 1.45x the bytes).
  * The band output is row-trimmed on-device before store: for partition
    group g (16 partitions), only band rows t in [g, g+9) are useful, and
    that slice is uniform across the group -> 8 plain partition-sliced DMAs
    per bx-column. Cuts output DMA from 384 to 216 elems/position (11.8MB ->
    6.6MB) with 576B contiguous runs.
Host does the remaining per-partition diagonal gather (u = j+dx), which no
lockstep engine can do on-chip.

fp16 in/out: full-rate on the PE (fp32 PSUM accumulation), half DMA volume.
Error vs the fp32 reference is ~1e-3 relative.
"""

import numpy as np

C, H, W = 256, 96, 160
PAD = 4
NG = 9  # displacement grid width (2*4+1)
Q = NG * NG  # 81 output channels
GB, BB = 8, 16  # stationary block: GB rows x BB cols = 128 positions
TT, UU = GB + 2 * PAD, BB + 2 * PAD  # moving block: 16 rows x 24 cols
NBY, NBX = H // GB, W // BB  # 12 x 10 = 120 blocks
NBLK = NBY * NBX
HP, WP = H + 2 * PAD, W + 2 * PAD
N_CORES = 8

_CACHE = {}


def _build_bass(reps=1):
    import contextlib

    import concourse.bass as bass  # noqa: F401
    import concourse.mybir as mybir
    import concourse.tile as tile
    from concourse import bacc

    fp16 = mybir.dt.float16
    fp32 = mybir.dt.float32

    nc = bacc.Bacc("TRN2", target_bir_lowering=False, debug=False)

    # d1b: data1 tiled into exact 8x16 blocks, bx-major -> [C, NBX, NBY, 128]
    # d2p: fully zero-padded data2 -> [C, HP, WP]
    d1b = nc.dram_tensor(
        "d1b", [C, NBX, NBY, 128], fp16, kind="ExternalInput"
    ).ap()
    d2p = nc.dram_tensor("d2p", [C, HP, WP], fp16, kind="ExternalInput").ap()
    # bands: trimmed band tiles [bx, g, j, t'=dy span 9, by, u=24]
    bands = nc.dram_tensor(
        "bands", [NBX, GB, 16, NG, NBY, UU], fp16, kind="ExternalOutput"
    ).ap()

    d1r = d1b.rearrange("(k p) bx by m -> p k bx by m", p=128)
    d2r = d2p.rearrange("(k p) y x -> p k y x", p=128)

    with tile.TileContext(nc) as tc:
        with (
            tc.tile_pool(name="in1", bufs=4) as in1_pool,
            tc.tile_pool(name="in2", bufs=2) as in2_pool,
            tc.tile_pool(name="ps", bufs=8, space="PSUM") as ps_pool,
            tc.tile_pool(name="ob", bufs=3) as ob_pool,
        ):
            n_phase = 2 if reps > 1 else 1
            loop = (
                tc.For_i(0, reps // n_phase, 1)
                if reps > 1
                else contextlib.nullcontext()
            )
            with loop:
                for _phase in range(n_phase):
                    # Whole padded data2 image; double-buffered across the
                    # 2-phase unroll so next rep's load overlaps compute.
                    d2_sb = in2_pool.tile([128, 2, HP, WP], fp16, tag="d2sb")
                    nc.sync.dma_start(out=d2_sb[:], in_=d2r)
                    for bx in range(NBX):
                        x0 = bx * BB
                        d1_sb = in1_pool.tile([128, 2, NBY, 128], fp16, tag="d1sb")
                        nc.gpsimd.dma_start(out=d1_sb[:], in_=d1r[:, :, bx])
                        ob = ob_pool.tile([128, TT, NBY, UU], fp16, tag="ob")
                        for by in range(NBY):
                            y0 = by * GB
                            ps = ps_pool.tile([128, TT, UU], fp32)
                            for k in range(2):
                                lhsT = d1_sb[:, k, by, :]
                                rhs = d2_sb[:, k, y0 : y0 + TT, x0 : x0 + UU]
                                nc.tensor.matmul(
                                    ps[:], lhsT, rhs, start=(k == 0), stop=(k == 1)
                                )
                            if by % 2 == 0:
                                nc.scalar.mul(ob[:, :, by, :], ps[:], 1.0 / C)
                            else:
                                nc.vector.tensor_scalar_mul(
                                    ob[:, :, by, :], ps[:], 1.0 / C
                                )
                        # Row-trim: partition group g only needs band rows
                        # t in [g, g+9). Uniform slice per group -> plain DMA.
                        for g in range(GB):
                            eng = nc.scalar if g % 2 == 0 else nc.gpsimd
                            eng.dma_start(
                                out=bands[bx, g],
                                in_=ob[16 * g : 16 * g + 16, g : g + NG, :, :],
                            )

    nc.compile()
    return nc


def _get_nc(reps=1):
    key = ("nc", reps)
    if key not in _CACHE:
        _CACHE[key] = _build_bass(reps)
    return _CACHE[key]


def _extract(bands_arr):
    """[NBX, GB, 16, NG, NBY, UU] fp16 trimmed bands -> [Q, H, W] fp32."""
    x = np.asarray(bands_arr)
    # sliding windows of width 9 along u: [NBX, GB, 16, NG, NBY, 16, NG]
    sw = np.lib.stride_tricks.sliding_window_view(x, NG, axis=5)
    jr = np.arange(16)
    # pick u0 = j per partition j -> [16(j), NBX, GB, NG(dy), NBY, NG(dx)]
    a = sw[:, :, jr, :, :, jr, :]
    # -> [dy, dx, by, g, bx, j] -> [Q, H, W]
    out = a.transpose(3, 5, 4, 2, 1, 0).reshape(Q, H, W)
    return out.astype(np.float32)


def prepare_inputs(data1, data2):
    """Full [N,C,H,W] fp32 inputs -> per-core in_maps (pre-blocked fp16)."""
    d1h = np.asarray(data1, dtype=np.float16)
    d2t = np.pad(
        np.asarray(data2, dtype=np.float16),
        ((0, 0), (0, 0), (PAD, PAD), (PAD, PAD)),
    )  # [N, C, HP, WP]
    # [N, C, H, W] -> [N, C, NBY, GB, NBX, BB] -> bx-major [N, C, NBX, NBY, 128]
    d1t = (
        d1h.reshape(N_CORES, C, NBY, GB, NBX, BB)
        .transpose(0, 1, 4, 2, 3, 5)
        .reshape(N_CORES, C, NBX, NBY, 128)
    )
    return [
        {
            "d1b": np.ascontiguousarray(d1t[i]),
            "d2p": np.ascontiguousarray(d2t[i]),
        }
        for i in range(N_CORES)
    ]


def _get_runner(reps=1):
    """Cached jit'd shard_map executable: f(*concat_inputs) -> concat outputs.

    Modeled on concourse.bass2jax.run_bass_via_pjrt, but built once and
    reusable so repeated kernel() calls (and timing loops) skip re-tracing.
    """
    rkey = ("runner", reps)
    if rkey in _CACHE:
        return _CACHE[rkey]

    import jax
    from jax.sharding import Mesh, PartitionSpec
    from jax.experimental.shard_map import shard_map
    import concourse.mybir as mybir
    from concourse import bass2jax

    bass2jax.install_neuronx_cc_hook()
    nc = _get_nc(reps)

    partition_name = nc.partition_id_tensor.name if nc.partition_id_tensor else None
    in_names, out_names, out_avals = [], [], []
    for alloc in nc.m.functions[0].allocations:
        if not isinstance(alloc, mybir.MemoryLocationSet):
            continue
        name = alloc.memorylocations[0].name
        if alloc.kind == "ExternalInput":
            if name != partition_name:
                in_names.append(name)
        elif alloc.kind == "ExternalOutput":
            out_names.append(name)
            out_avals.append(
                jax.core.ShapedArray(
                    tuple(alloc.tensor_shape), mybir.dt.np(alloc.dtype)
                )
            )
    n_params = len(in_names)
    all_in_names = in_names + out_names
    if partition_name is not None:
        all_in_names = all_in_names + [partition_name]

    def _body(*args):
        operands = list(args)
        if partition_name is not None:
            operands.append(bass2jax.partition_id_tensor())
        outs = bass2jax._bass_exec_p.bind(
            *operands,
            out_avals=tuple(out_avals),
            in_names=tuple(all_in_names),
            out_names=tuple(out_names),
            lowering_input_output_aliases=(),
            sim_require_finite=True,
            sim_require_nnan=True,
            nc=nc,
        )
        return tuple(outs)

    devices = jax.devices()[:N_CORES]
    mesh = Mesh(np.asarray(devices), ("core",))
    n_outs = len(out_names)
    sharded = jax.jit(
        shard_map(
            _body,
            mesh=mesh,
            in_specs=(PartitionSpec("core"),) * (n_params + n_outs),
            out_specs=(PartitionSpec("core"),) * n_outs,
            check_rep=False,
        ),
        keep_unused=True,
    )
    runner = {
        "fn": sharded,
        "in_names": in_names,
        "out_names": out_names,
        "out_avals": out_avals,
        "mesh": mesh,
    }
    _CACHE[rkey] = runner
    return runner


def run_hw(in_maps):
    """Execute on 8 cores; returns list of per-core {name: np.ndarray}."""
    r = _get_runner()
    concat_in = [
        np.concatenate([m[name] for m in in_maps], axis=0) for name in r["in_names"]
    ]
    concat_zeros = [
        np.zeros((N_CORES * a.shape[0], *a.shape[1:]), a.dtype)
        for a in r["out_avals"]
    ]
    out_arrs = r["fn"](*concat_in, *concat_zeros)
    return [
        {
            name: np.asarray(out_arrs[i]).reshape(
                N_CORES, *r["out_avals"][i].shape
            )[c]
            for i, name in enumerate(r["out_names"])
        }
        for c in range(N_CORES)
    ]


def kernel(data1, data2):
    in_maps = prepare_inputs(data1, data2)
    results = run_hw(in_maps)
    out = np.stack([_extract(r["bands"]) for r in results])
    return out.astype(np.float32)
